# revision 3
# baseline (speedup 1.0000x reference)
"""Trainium2 Bass kernel for a 3-type heterogeneous GraphSAGE GNN.

Full-input contract: kernel(**inputs) takes the unsharded numpy inputs and
returns the full [300000, 2] output. Internally:
  - Nodes are relabeled so each of the 8 cores owns a contiguous range of
    37632 padded nodes (12544 per type); edges are sharded by dst owner.
  - Per core, edges are sorted by (src block of 32768, dst) and padded into
    a schedule of 128-edge chunks that is *uniform across cores* (the NEFF
    is SPMD — one program, per-core data).
  - Aggregation: dma_gather pulls x[src] rows (256B) from a replicated
    x_full in DRAM; a 0/1 one-hot [128 edges x 128 dsts] built on DVE via
    is_equal(iota, dstrel) turns segment-sum into PE matmuls accumulated
    in PSUM; banks are flushed into an SBUF accumulator; mean = agg * winv.
  - x_full is refreshed between layers with an AllGather collective.
"""

import numpy as np

import concourse.bass as bass
import concourse.bacc as bacc
import concourse.mybir as mybir
import concourse.tile as tile
from concourse.masks import make_identity
from concourse.bass_utils import run_bass_kernel_spmd

F32 = mybir.dt.float32
I16 = mybir.dt.int16

FULL_CFG = dict(type_size=100000, E=4800000, cores=8, blk=32768,
                strip_chunks=32, h=64)


def derive(cfg):
    cores = cfg["cores"]
    seg = cfg["type_size"] // cores          # real nodes per (core, type)
    assert seg * cores == cfg["type_size"]
    segp = -(-seg // 128) * 128              # padded to tile multiple
    npc = 3 * segp                           # nodes per core (padded)
    nptot = cores * npc
    tiles = npc // 128
    groups = tiles                           # 128-dst groups per core
    nblk = -(-nptot // cfg["blk"])
    d = dict(cfg)
    d.update(seg=seg, segp=segp, npc=npc, nptot=nptot, tiles=tiles,
             groups=groups, nblk=nblk)
    return d


def node_perm(d):
    """perm_of_orig[j] = padded-global id of original node j."""
    ts, seg, segp, npc = d["type_size"], d["seg"], d["segp"], d["npc"]
    j = np.arange(ts)
    core = j // seg
    local = j % seg
    parts = [core * npc + t * segp + local for t in range(3)]
    return np.concatenate(parts)


class Sched:
    pass


def plan(d, edge_index):
    """Build the uniform schedule + per-core edge data arrays."""
    cores, npc, nptot, blk = d["cores"], d["npc"], d["nptot"], d["blk"]
    groups, nblk, sc = d["groups"], d["nblk"], d["strip_chunks"]

    perm = node_perm(d)
    src_p = perm[np.asarray(edge_index[0], dtype=np.int64)]
    dst_p = perm[np.asarray(edge_index[1], dtype=np.int64)]

    deg = np.bincount(dst_p, minlength=nptot).astype(np.float64)
    winv_full = (1.0 / np.maximum(deg, 1.0)).astype(np.float32)

    # per-core sorted edge arrays + per-(block, group) counts
    core_of = dst_p // npc
    per_core = []
    counts = np.zeros((cores, nblk, groups), np.int64)
    for c in range(cores):
        m = core_of == c
        es = src_p[m]
        ed = dst_p[m] - c * npc
        b = es // blk
        order = np.lexsort((ed, b))
        es, ed, b = es[order], ed[order], b[order]
        g = ed // 128
        np.add.at(counts[c], (b, g), 1)
        per_core.append((es, ed, b, g))

    nch = np.maximum(1, -(-counts.max(axis=0) // 128))  # [nblk, groups]

    # chunk list in program order: for b, for g, k in range(nch[b,g])
    chunk_b, chunk_g, chunk_start, chunk_stop = [], [], [], []
    for b in range(nblk):
        for g in range(groups):
            n = int(nch[b, g])
            for k in range(n):
                chunk_b.append(b)
                chunk_g.append(g)
                chunk_start.append(k == 0)
                chunk_stop.append(k == n - 1)
    nchunks = len(chunk_b)
    chunk_b = np.array(chunk_b)
    chunk_g = np.array(chunk_g)

    # strips: cut chunk list per-block at strip_chunks boundary
    strips = []  # (b, c0, n, idx_col_off)
    idx_off = 0
    pos = 0
    for b in range(nblk):
        nb = int(nch[b].sum())
        o = 0
        while o < nb:
            n = min(sc, nb - o)
            strips.append((b, pos + o, n, idx_off))
            idx_off += n * 8
            o += n
        pos += nb
    idx_cols = idx_off
    strip_of_chunk = np.zeros(nchunks, np.int64)
    strip_c0 = np.zeros(nchunks, np.int64)
    for si, (b, c0, n, io) in enumerate(strips):
        strip_of_chunk[c0:c0 + n] = si
        strip_c0[c0:c0 + n] = c0

    # flush ops: after the last chunk of each (b, bank-of-8-groups)
    # op list: interleave strips/mms/flushes in program order
    ops = []
    cpos = 0
    for si, (b, c0, n, io) in enumerate(strips):
        ops.append(("strip", si))
        for k in range(n):
            ci = c0 + k
            ops.append(("mm", si, k, int(chunk_g[ci]),
                        bool(chunk_start[ci]), bool(chunk_stop[ci])))
            nxt = ci + 1
            if chunk_stop[ci]:
                g = int(chunk_g[ci])
                bank_end = (nxt == nchunks or chunk_b[nxt] != chunk_b[ci]
                            or chunk_g[nxt] // 8 != g // 8)
                if bank_end:
                    g_lo = (g // 8) * 8
                    g_hi = min(g_lo + 7, groups - 1)
                    ops.append(("flush", g_lo, g_hi))
        cpos += n

    # ---- per-core data arrays ----
    # padded slots: cell (b,g) occupies nch[b,g]*128 consecutive slots
    cell_nslots = (nch * 128).ravel()                    # [nblk*groups]
    cell_pad_start = np.concatenate([[0], np.cumsum(cell_nslots)])
    total_slots = int(cell_pad_start[-1])
    assert total_slots == nchunks * 128
    slot = np.arange(total_slots)
    cell_of_slot = np.searchsorted(cell_pad_start, slot, "right") - 1
    within = slot - cell_pad_start[cell_of_slot]

    # chunk index of each slot & strip-local edge index
    ch_of_slot = slot // 128
    strip_local = (ch_of_slot - strip_c0[ch_of_slot]) * 128 + slot % 128
    idx_col = np.array([strips[s][3] for s in strip_of_chunk[ch_of_slot]]) \
        + strip_local // 16
    idx_row = strip_local % 16
    slot_b = chunk_b[ch_of_slot]
    slot_g = chunk_g[ch_of_slot]

    idx_all = np.zeros((cores, 128, idx_cols), np.int16)
    dstrel_all = np.full((cores, 128, nchunks), -1.0, np.float32)
    for c in range(cores):
        es, ed, b, g = per_core[c]
        ccounts = counts[c].ravel()
        cell_start = np.concatenate([[0], np.cumsum(ccounts)])
        real = within < ccounts[cell_of_slot]
        src_idx = cell_start[cell_of_slot] + np.minimum(
            within, np.maximum(ccounts[cell_of_slot] - 1, 0))
        esv = np.where(real, es[np.minimum(src_idx, len(es) - 1)]
                       if len(es) else 0, 0)
        edv = np.where(real, ed[np.minimum(src_idx, len(ed) - 1)]
                       if len(ed) else 0, -1)
        rel = np.where(real, esv - slot_b * blk, 0).astype(np.int64)
        assert rel.min() >= 0 and rel.max() < blk
        drel = np.where(real, edv - slot_g * 128, -1.0).astype(np.float32)
        for r in range(8):
            idx_all[c, idx_row + 16 * r, idx_col] = rel.astype(np.int16)
        dstrel_all[c, slot % 128, ch_of_slot] = drel

    s = Sched()
    s.d = d
    s.perm = perm
    s.strips = strips
    s.ops = ops
    s.nchunks = nchunks
    s.idx_cols = idx_cols
    s.winv_full = winv_full
    s.idx_all = idx_all
    s.dstrel_all = dstrel_all
    return s


def core_inputs(s, x_individual, x_company, x_trust,
                W_ind, b_ind, W_com, b_com, W_tru, b_tru,
                W1l, W1r, b1, W2l, W2r, b2, Wc1, bc1, Wc2, bc2):
    d = s.d
    cores, seg, segp, npc, groups = \
        d["cores"], d["seg"], d["segp"], d["npc"], d["groups"]
    raws = [np.asarray(x_individual, np.float32),
            np.asarray(x_company, np.float32),
            np.asarray(x_trust, np.float32)]
    Ws = [np.asarray(W_ind, np.float32), np.asarray(W_com, np.float32),
          np.asarray(W_tru, np.float32)]
    bs = [np.asarray(b_ind, np.float32), np.asarray(b_com, np.float32),
          np.asarray(b_tru, np.float32)]
    h = d["h"]
    kenc = 49  # 48 padded features + ones column

    w_enc = []
    for t in range(3):
        w = np.zeros((kenc, h), np.float32)
        w[:Ws[t].shape[0], :] = Ws[t]
        w[48, :] = bs[t]
        w_enc.append(w)

    shared = {
        "w_enc0": w_enc[0], "w_enc1": w_enc[1], "w_enc2": w_enc[2],
        "w1l": np.asarray(W1l, np.float32), "w1r": np.asarray(W1r, np.float32),
        "w2l": np.asarray(W2l, np.float32), "w2r": np.asarray(W2r, np.float32),
        "wc1": np.asarray(Wc1, np.float32), "wc2": np.asarray(Wc2, np.float32),
        "b1_rep": np.tile(np.asarray(b1, np.float32)[None, :], (128, 1)),
        "b2_rep": np.tile(np.asarray(b2, np.float32)[None, :], (128, 1)),
        "bc1_rep": np.tile(np.asarray(bc1, np.float32)[None, :], (128, 1)),
        "bc2_rep": np.tile(np.asarray(bc2, np.float32)[None, :], (128, 1)),
        "iota_rep": np.tile(np.arange(128, dtype=np.float32)[None, :],
                            (128, 1)),
    }

    in_maps = []
    for c in range(cores):
        xraw = np.zeros((npc, kenc), np.float32)
        for t in range(3):
            r0 = t * segp
            xraw[r0:r0 + seg, :raws[t].shape[1]] = \
                raws[t][c * seg:(c + 1) * seg]
            xraw[r0:r0 + seg, 48] = 1.0
        winv = s.winv_full[c * npc:(c + 1) * npc] \
            .reshape(groups, 128).T.copy()
        m = dict(shared)
        m.update(xraw=xraw, idx=s.idx_all[c], dstrel=s.dstrel_all[c],
                 winv=winv)
        in_maps.append(m)
    return in_maps


def build_program(s):
    d = s.d
    cores, npc, nptot, blk = d["cores"], d["npc"], d["nptot"], d["blk"]
    tiles, groups, nblk, h = d["tiles"], d["groups"], d["nblk"], d["h"]
    slots = min(64, groups) if groups <= 64 else 64
    kenc = 49

    nc = bacc.Bacc("TRN2", target_bir_lowering=False, debug=False,
                   num_devices=cores, dynamic_dma_scratch_size=32768)

    di = {}
    def inp(name, shape, dt=F32):
        di[name] = nc.dram_tensor(name, list(shape), dt, kind="ExternalInput")
        return di[name]

    inp("xraw", [npc, kenc])
    inp("idx", [128, s.idx_cols], I16)
    inp("dstrel", [128, s.nchunks])
    inp("winv", [128, groups])
    inp("iota_rep", [128, 128])
    for t in range(3):
        inp(f"w_enc{t}", [kenc, h])
    inp("w1l", [h, h]); inp("w1r", [h, h]); inp("b1_rep", [128, h])
    inp("w2l", [h, h]); inp("w2r", [h, h]); inp("b2_rep", [128, h])
    inp("wc1", [h, 32]); inp("bc1_rep", [128, 32])
    inp("wc2", [32, 2]); inp("bc2_rep", [128, 2])
    out_d = nc.dram_tensor("out", [npc, 2], F32, kind="ExternalOutput")

    AG = "AllGather"
    ADD = mybir.AluOpType.add
    MUL = mybir.AluOpType.mult
    EQ = mybir.AluOpType.is_equal
    BYP = mybir.AluOpType.bypass
    RELU = mybir.ActivationFunctionType.Relu

    with tile.TileContext(nc) as tc:
        with tc.tile_pool(name="persist", bufs=1) as pp, \
             tc.tile_pool(name="dram", bufs=1, space="DRAM") as dramp:
            # constants to SBUF
            def csb(name, shape, dt=F32):
                t_ = pp.tile(list(shape), dt, tag=name)
                nc.sync.dma_start(t_[:], di[name].ap())
                return t_
            iota_sb = csb("iota_rep", [128, 128])
            winv_sb = csb("winv", [128, groups])
            wenc_sb = [csb(f"w_enc{t}", [kenc, h]) for t in range(3)]
            wl_sb = [csb("w1l", [h, h]), csb("w2l", [h, h])]
            wr_sb = [csb("w1r", [h, h]), csb("w2r", [h, h])]
            brep_sb = [csb("b1_rep", [128, h]), csb("b2_rep", [128, h])]
            wc1_sb = csb("wc1", [h, 32])
            bc1_sb = csb("bc1_rep", [128, 32])
            wc2_sb = csb("wc2", [32, 2])
            bc2_sb = csb("bc2_rep", [128, 2])
            ident = pp.tile([128, 128], F32, tag="ident")
            make_identity(nc, ident[:])

            agg = pp.tile([128, groups * h], F32, tag="agg")

            x_own0 = dramp.tile([npc, h], F32)
            x_own1 = dramp.tile([npc, h], F32)
            xf = [dramp.tile([nptot, h], F32, name="xf0"),
                  dramp.tile([nptot, h], F32, name="xf1")]
            x_own = [x_own0, x_own1]

            # ---------------- encoder ----------------
            with tc.tile_pool(name="encio", bufs=1) as pio, \
                 tc.tile_pool(name="enc", bufs=4) as pe, \
                 tc.tile_pool(name="encps", bufs=4, space="PSUM") as pse:
                xraw_sb = pio.tile([128, tiles * kenc], F32)
                nc.sync.dma_start(
                    xraw_sb[:].rearrange("p (t f) -> p t f", f=kenc),
                    di["xraw"].ap().rearrange("(t p) f -> p t f", p=128))
                seg_tiles = d["segp"] // 128
                for t in range(tiles):
                    wseg = wenc_sb[t // seg_tiles]
                    tp = pse.tile([64, 128], F32, tag="tp")
                    nc.tensor.transpose(
                        out=tp[:kenc, :],
                        in_=xraw_sb[:, t * kenc:(t + 1) * kenc],
                        identity=ident[:])
                    xrT = pe.tile([kenc, 128], F32, tag="xrT")
                    nc.scalar.copy(xrT[:], tp[:kenc, :])
                    ym = pse.tile([128, h], F32, tag="ym")
                    nc.tensor.matmul(out=ym[:], lhsT=xrT[:], rhs=wseg[:],
                                     start=True, stop=True)
                    nc.scalar.copy(agg[:, t * h:(t + 1) * h], ym[:])
                nc.sync.dma_start(
                    x_own0[:, :].rearrange("(t p) f -> p t f", p=128),
                    agg[:].rearrange("p (t f) -> p t f", f=h))
            nc.gpsimd.collective_compute(
                AG, BYP, replica_groups=[list(range(cores))],
                ins=[x_own0[:, :]], outs=[xf[0][:, :]])

            # ---------------- SAGE layers ----------------
            for L in range(2):
                with tc.tile_pool(name=f"sage{L}", bufs=2) as pa, \
                     tc.tile_pool(name=f"sageps{L}", bufs=1,
                                  space="PSUM") as psa:
                    nc.vector.memset(agg[:], 0.0)
                    psum_agg = psa.tile([128, slots * h], F32)
                    cur = {}
                    for op in s.ops:
                        if op[0] == "strip":
                            si = op[1]
                            b, c0, n, ioff = s.strips[si]
                            idx_sb = pa.tile([128, n * 8], I16, tag="idx")
                            nc.sync.dma_start(
                                idx_sb[:],
                                di["idx"].ap()[:, ioff:ioff + n * 8])
                            dst_sb = pa.tile([128, n], F32, tag="dst")
                            nc.sync.dma_start(
                                dst_sb[:],
                                di["dstrel"].ap()[:, c0:c0 + n])
                            msgs = pa.tile([128, n * h], F32, tag="msgs")
                            oh = pa.tile([128, n * 128], F32, tag="oh")
                            rows = min(blk, nptot - b * blk)
                            nc.gpsimd.dma_gather(
                                out_ap=msgs[:].rearrange(
                                    "p (c f) -> p c f", f=h),
                                in_ap=xf[L][b * blk:b * blk + rows, :],
                                idxs_ap=idx_sb[:],
                                num_idxs=n * 128, num_idxs_reg=n * 128,
                                elem_size=h, single_packet=False)
                            nc.vector.tensor_tensor(
                                out=oh[:].rearrange("p (c w) -> p c w",
                                                    w=128),
                                in0=dst_sb[:][:, :, None].to_broadcast(
                                    [128, n, 128]),
                                in1=iota_sb[:][:, None, :].to_broadcast(
                                    [128, n, 128]),
                                op=EQ)
                            cur = dict(msgs=msgs, oh=oh)
                        elif op[0] == "mm":
                            _, si, k, g, st, sp = op
                            sl = g % slots
                            nc.tensor.matmul(
                                out=psum_agg[:, sl * h:(sl + 1) * h],
                                lhsT=cur["oh"][:, k * 128:(k + 1) * 128],
                                rhs=cur["msgs"][:, k * h:(k + 1) * h],
                                start=st, stop=sp)
                        else:  # flush
                            _, g_lo, g_hi = op
                            sl = g_lo % slots
                            w = (g_hi - g_lo + 1) * h
                            nc.vector.tensor_tensor(
                                out=agg[:, g_lo * h:g_lo * h + w],
                                in0=agg[:, g_lo * h:g_lo * h + w],
                                in1=psum_agg[:, sl * h:sl * h + w],
                                op=ADD)
                    nc.vector.tensor_tensor(
                        out=agg[:].rearrange("p (g f) -> p g f", f=h),
                        in0=agg[:].rearrange("p (g f) -> p g f", f=h),
                        in1=winv_sb[:][:, :, None].to_broadcast(
                            [128, groups, h]),
                        op=MUL)

                # post-aggregation: y = relu(mean@Wl + x@Wr + b)
                with tc.tile_pool(name=f"post{L}io", bufs=1) as pio, \
                     tc.tile_pool(name=f"post{L}", bufs=4) as ppo, \
                     tc.tile_pool(name=f"post{L}ps", bufs=2,
                                  space="PSUM") as psp:
                    xin_sb = pio.tile([128, tiles * h], F32)
                    nc.sync.dma_start(
                        xin_sb[:].rearrange("p (t f) -> p t f", f=h),
                        x_own[L][:, :].rearrange("(t p) f -> p t f", p=128))
                    for t in range(tiles):
                        tp1 = psp.tile([64, 128], F32, tag="tp1")
                        nc.tensor.transpose(
                            out=tp1[:h, :], in_=agg[:, t * h:(t + 1) * h],
                            identity=ident[:])
                        aggT = ppo.tile([h, 128], F32, tag="aggT")
                        nc.scalar.copy(aggT[:], tp1[:h, :])
                        tp2 = psp.tile([64, 128], F32, tag="tp2")
                        nc.tensor.transpose(
                            out=tp2[:h, :],
                            in_=xin_sb[:, t * h:(t + 1) * h],
                            identity=ident[:])
                        xT = ppo.tile([h, 128], F32, tag="xT")
                        nc.scalar.copy(xT[:], tp2[:h, :])
                        ym = psp.tile([128, h], F32, tag="ym")
                        nc.tensor.matmul(out=ym[:], lhsT=aggT[:],
                                         rhs=wl_sb[L][:],
                                         start=True, stop=False)
                        nc.tensor.matmul(out=ym[:], lhsT=xT[:],
                                         rhs=wr_sb[L][:],
                                         start=False, stop=True)
                        tmp = ppo.tile([128, h], F32, tag="tmp")
                        nc.vector.tensor_tensor(out=tmp[:], in0=ym[:],
                                                in1=brep_sb[L][:], op=ADD)
                        nc.scalar.activation(
                            out=agg[:, t * h:(t + 1) * h], in_=tmp[:],
                            func=RELU)
                    if L == 0:
                        nc.sync.dma_start(
                            x_own1[:, :].rearrange("(t p) f -> p t f",
                                                   p=128),
                            agg[:].rearrange("p (t f) -> p t f", f=h))
                if L == 0:
                    nc.gpsimd.collective_compute(
                        AG, BYP, replica_groups=[list(range(cores))],
                        ins=[x_own1[:, :]], outs=[xf[1][:, :]])

            # ---------------- classifier ----------------
            with tc.tile_pool(name="clsio", bufs=1) as pio, \
                 tc.tile_pool(name="cls", bufs=4) as pc, \
                 tc.tile_pool(name="clsps", bufs=2, space="PSUM") as psc:
                outb = pio.tile([128, tiles * 2], F32)
                for t in range(tiles):
                    tp1 = psc.tile([64, 128], F32, tag="tp1")
                    nc.tensor.transpose(
                        out=tp1[:h, :], in_=agg[:, t * h:(t + 1) * h],
                        identity=ident[:])
                    x2T = pc.tile([h, 128], F32, tag="x2T")
                    nc.scalar.copy(x2T[:], tp1[:h, :])
                    hps = psc.tile([128, 32], F32, tag="hps")
                    nc.tensor.matmul(out=hps[:], lhsT=x2T[:], rhs=wc1_sb[:],
                                     start=True, stop=True)
                    htmp = pc.tile([128, 32], F32, tag="htmp")
                    nc.vector.tensor_tensor(out=htmp[:], in0=hps[:],
                                            in1=bc1_sb[:], op=ADD)
                    hsb = pc.tile([128, 32], F32, tag="hsb")
                    nc.scalar.activation(out=hsb[:], in_=htmp[:], func=RELU)
                    tp2 = psc.tile([64, 128], F32, tag="tp2")
                    nc.tensor.transpose(out=tp2[:32, :], in_=hsb[:],
                                        identity=ident[:])
                    hT = pc.tile([32, 128], F32, tag="hT")
                    nc.scalar.copy(hT[:], tp2[:32, :])
                    ops_ = psc.tile([128, 2], F32, tag="ops")
                    nc.tensor.matmul(out=ops_[:], lhsT=hT[:], rhs=wc2_sb[:],
                                     start=True, stop=True)
                    nc.vector.tensor_tensor(out=outb[:, t * 2:(t + 1) * 2],
                                            in0=ops_[:], in1=bc2_sb[:],
                                            op=ADD)
                nc.sync.dma_start(
                    out_d.ap().rearrange("(t p) f -> p t f", p=128),
                    outb[:].rearrange("p (t f) -> p t f", f=2))

    nc.compile()
    return nc


def run(cfg, inputs, trace=False):
    d = derive(cfg)
    s = plan(d, inputs["edge_index"])
    in_maps = core_inputs(
        s, **{k: v for k, v in inputs.items() if k != "edge_index"})
    nc = build_program(s)
    res = run_bass_kernel_spmd(nc, in_maps, core_ids=list(range(d["cores"])),
                               trace=trace)
    outs = [res.results[c]["out"] for c in range(d["cores"])]
    out_full = np.concatenate(outs, axis=0)  # [nptot, 2]
    final = out_full[s.perm]                 # original node order
    return final.astype(np.float32), res


def kernel(**inputs):
    out, _ = run(FULL_CFG, inputs)
    return out

